# revision 43
# baseline (speedup 1.0000x reference)
"""Trainium2 Bass kernel for nn_MessageAggregationAttention.

Shards B=256 graphs across 8 NeuronCores (32 graphs each). The host does
all data *layout* (gather / pad / transpose / cast); every FLOP of the
model (projections, attention, FFN) runs on device.

Shape specialization: graph sizes are known at kernel() time, so each
core sorts its graphs by key-count and rank-i graphs across cores share
slot i, whose capacities are the across-core maxima: QC[i] query slots
(multiple of 4, <= 96 instead of a flat 96 pad) and KT[i] 128-key tiles
(2 or 3 instead of a flat 3). This removes ~25% of the padded attention
work. The program is built once per shape signature and reused.

Host prep per core:
  - xqr/xqbf [128, QS2]: Q token slab, feature-major (f32 with the
    out-proj bias + folded Wo@bv added for the residual spine; bf16 copy
    for the Q projection).
  - xkT [128, KS2]: incoming-message rows gathered on host
    (edge_attr[incoming_edges_list]), zero-padded per slot, transposed,
    bf16 — replaces 96 serial INDIRECT1D gathers (~105us of GpSimd
    descriptor time) with plain DMA.
  - The key bias bk is dropped exactly (softmax is invariant to the
    per-query shift q.bk); zero-padded K columns then give logits==0,
    exp==1, so the denominator over-counts by exactly npad, which the
    kernel subtracts (no mask table at all).

Device per slot (all matmuls bf16, f32 PSUM), software-pipelined in
waves (kv | logits+exp | ctx+den | norm+outproj, 5 waves deep) so the
in-order engine queues never wait on same-wave work:
  - K/V projections from the resident xkT slab.
  - Logits: zero-blocked qTz (full-128 contraction) per key tile; Exp on
    Scalar with no bias operand.
  - Denominator via ones[128,32] matmuls whose replicated output doubles
    as the partition-broadcast for normalization.
  - Out-proj, residual add; FFN blocks interleave into the wave loop as
    their columns finalize, and the output streams out per block.
"""

import math

import ml_dtypes
import numpy as np

import concourse.bass as bass
import concourse.mybir as mybir
from concourse import bacc
from concourse.bass_utils import run_bass_kernel_spmd
from concourse.tile import TileContext

B, E, M, H, NH = 256, 16384, 65536, 128, 4
HD = H // NH               # 32
LQ, LK = 96, 384           # hard capacity ceilings per graph
NCORES = 8
G = B // NCORES            # 32 graphs per core

f32 = mybir.dt.float32
bf16 = mybir.dt.bfloat16

AFT = mybir.ActivationFunctionType
ALU = mybir.AluOpType

LAST_RESULTS = None
TRACE = False
TRACE_KW = {}


def _build_program(QC, KT):
    QOFF = [0]
    for q in QC:
        QOFF.append(QOFF[-1] + q)
    KOFF = [0]
    for k in KT:
        KOFF.append(KOFF[-1] + 128 * k)
    QS2, KS2 = QOFF[-1], KOFF[-1]

    # FFN blocks of <=512 cols; the last one split in two to drain faster
    blocks = []
    c = 0
    while c < QS2:
        blocks.append((c, min(c + 512, QS2)))
        c = min(c + 512, QS2)
    b0, b1 = blocks.pop()
    if b1 - b0 > 256:
        mid = b0 + ((b1 - b0) // 2 + 3) // 4 * 4
        blocks.append((b0, mid))
        blocks.append((mid, b1))
    else:
        blocks.append((b0, b1))
    # earliest wave per block: its last slot s finishes norm at wave s+5
    ffn_a, ffn_b = {}, {}
    prev_wa = -10
    for (c0, c1) in blocks:
        smax = max(s for s in range(G) if QOFF[s] < c1)
        wa = max(smax + 6, prev_wa + 2)
        prev_wa = wa
        ffn_a.setdefault(wa, []).append((c0, c1))
        ffn_b.setdefault(wa + 1, []).append((c0, c1))
    tail_keys = set(blocks[-2:])
    n_waves = max(G + 7, max(ffn_b) + 1)

    nc = bacc.Bacc("TRN2")

    xkT_d = nc.dram_tensor("xkT", [H, KS2], bf16, kind="ExternalInput")
    xqbf_d = nc.dram_tensor("xqbf", [H, QS2], bf16, kind="ExternalInput")
    xqr_d = nc.dram_tensor("xqr", [H, QS2], f32, kind="ExternalInput")
    wqTz_d = nc.dram_tensor("wqTz", [H, 4 * H], bf16, kind="ExternalInput")
    wkT_d = nc.dram_tensor("wkT", [H, H], bf16, kind="ExternalInput")
    wvT_d = nc.dram_tensor("wvT", [H, H], bf16, kind="ExternalInput")
    woT_d = nc.dram_tensor("woT", [H, H], bf16, kind="ExternalInput")
    w1T_d = nc.dram_tensor("w1T", [H, 2 * H], bf16, kind="ExternalInput")
    w2T_d = nc.dram_tensor("w2T", [2 * H, H], bf16, kind="ExternalInput")
    bq_d = nc.dram_tensor("bqz", [H, 4], f32, kind="ExternalInput")
    b1_d = nc.dram_tensor("b1c", [H, 2], f32, kind="ExternalInput")
    b2_d = nc.dram_tensor("b2c", [H, 1], f32, kind="ExternalInput")
    nnp_d = nc.dram_tensor("negnp", [H, G], f32, kind="ExternalInput")

    out_d = nc.dram_tensor("out", [H, QS2], f32, kind="ExternalOutput")

    with TileContext(nc) as tc:
        with (
            tc.tile_pool(name="const", bufs=1) as constp,
            tc.tile_pool(name="kv", bufs=5) as kvp,
            tc.tile_pool(name="exp", bufs=6) as expp,
            tc.tile_pool(name="sm", bufs=3) as smp,
            tc.tile_pool(name="ffn", bufs=2) as ffnp,
            tc.tile_pool(name="ps_big", bufs=2, space="PSUM") as ps_bigp,
            tc.tile_pool(name="ps_kv", bufs=1, space="PSUM") as ps_kvp,
            tc.tile_pool(name="ps_lg", bufs=2, space="PSUM") as ps_lgp,
            tc.tile_pool(name="ps_att", bufs=2, space="PSUM") as ps_attp,
        ):
            ones32 = constp.tile([128, 32], bf16)
            nc.vector.memset(ones32[:], 1.0)

            def _load(shape, dram, dt=f32):
                t = constp.tile(shape, dt, tag=dram.name, name=dram.name + "_sb")
                nc.sync.dma_start(out=t[:], in_=dram[:])
                return t

            # weights needed in the first waves load first; FFN weights
            # queue behind the xkT chunks below
            wqTz = _load([H, 4 * H], wqTz_d, bf16)
            wkT = _load([H, H], wkT_d, bf16)
            wvT = _load([H, H], wvT_d, bf16)
            woT = _load([H, H], woT_d, bf16)
            bqz = _load([H, 4], bq_d)
            negnp = _load([H, G], nnp_d)

            # Input slabs: spread dma_start descriptor generation across
            # engine queues (~0.6us serial per call) and order chunks so
            # wave-0 consumers land first.
            xkT = constp.tile([128, KS2], bf16, tag="xkT", name="xkT")
            xqbf = constp.tile([128, QS2], bf16, tag="xqbf", name="xqbf")
            xqr = constp.tile([128, QS2], f32, tag="xqr", name="xqr")

            def _chunk(eng, dst, src, c0, c1):
                if c1 > c0:
                    eng.dma_start(out=dst[:, c0:c1], in_=src[:, c0:c1])

            q1 = min(512, QS2)
            q2 = min(1792, QS2)
            _chunk(nc.scalar, xqbf, xqbf_d, 0, 128)
            _chunk(nc.scalar, xqbf, xqbf_d, 128, q1)
            _chunk(nc.scalar, xkT, xkT_d, 0, KOFF[2])
            _chunk(nc.gpsimd, xkT, xkT_d, KOFF[2], KOFF[6])
            _chunk(nc.gpsimd, xqbf, xqbf_d, q1, q2)
            for s0 in range(6, G, 5):
                _chunk(nc.sync, xkT, xkT_d, KOFF[s0], KOFF[min(s0 + 5, G)])
            _chunk(nc.sync, xqbf, xqbf_d, q2, QS2)
            _chunk(nc.sync, xqr, xqr_d, 0, QS2 // 2 // 4 * 4)
            _chunk(nc.gpsimd, xqr, xqr_d, QS2 // 2 // 4 * 4, QS2)

            w1T = _load([H, 2 * H], w1T_d, bf16)
            w2T_a = constp.tile([128, H], bf16, tag="w2Ta")
            w2T_b = constp.tile([128, H], bf16, tag="w2Tb")
            nc.sync.dma_start(out=w2T_a[:], in_=w2T_d[0:128, :])
            nc.sync.dma_start(out=w2T_b[:], in_=w2T_d[128:256, :])
            b1c = _load([H, 2], b1_d)
            b2c = _load([H, 1], b2_d)

            qTz = constp.tile([128, 4, QS2], bf16, tag="qTz", name="qTz")
            ar = constp.tile([128, QS2], f32, tag="ar", name="ar")

            def emit_qproj(blk, ranges=None):
                if ranges is None:
                    if blk == 0:
                        ranges = [(0, 128), (128, min(512, QS2))]
                    else:
                        c0 = blk * 512
                        if c0 >= QS2:
                            return
                        ranges = [(c0, min(c0 + 512, QS2))]
                for c0, c1 in ranges:
                    _emit_qproj_range(c0, c1)

            def _emit_qproj_range(c0, c1):
                sl = slice(c0, c1)
                n = c1 - c0
                for h in range(4):
                    psq = ps_bigp.tile([128, 512], f32, tag="big", name="psq")
                    nc.tensor.matmul(
                        out=psq[:, 0:n], lhsT=wqTz[:, h * 128 : (h + 1) * 128],
                        rhs=xqbf[:, sl], start=True, stop=True,
                        skip_group_check=True,
                    )
                    if h < 2:
                        nc.scalar.activation(
                            out=qTz[:, h, sl], in_=psq[:, 0:n],
                            func=AFT.Identity, bias=bqz[:, h : h + 1],
                        )
                    else:
                        nc.vector.tensor_scalar_add(
                            out=qTz[:, h, sl], in0=psq[:, 0:n],
                            scalar1=bqz[:, h : h + 1],
                        )

            kT_g, v_g, ex_g, exs_g, att_g = {}, {}, {}, {}, {}

            def emit_kv(g):
                kw = 128 * KT[g]
                ksl = slice(KOFF[g], KOFF[g + 1])
                psk = ps_kvp.tile([128, LK], f32, tag="psk", name="psk")
                nc.tensor.matmul(
                    out=psk[:, 0:kw], lhsT=wkT[:], rhs=xkT[:, ksl],
                    start=True, stop=True, skip_group_check=True,
                )
                kT = kvp.tile([128, LK], bf16, tag="kT", name="kT", bufs=6)
                nc.scalar.activation(
                    out=kT[:, 0:kw], in_=psk[:, 0:kw], func=AFT.Identity)
                psv = ps_kvp.tile([128, LK], f32, tag="psv", name="psv")
                for t in range(KT[g]):
                    nc.tensor.matmul(
                        out=psv[:, t * 128 : (t + 1) * 128],
                        lhsT=xkT[:, KOFF[g] + t * 128 : KOFF[g] + (t + 1) * 128],
                        rhs=wvT[:],
                        start=True, stop=True, skip_group_check=True,
                    )
                v = kvp.tile([128, LK], bf16, tag="v", name="v", bufs=8)
                nc.vector.tensor_copy(out=v[:, 0:kw], in_=psv[:, 0:kw])
                kT_g[g] = kT
                v_g[g] = v

            def emit_lgx(g):
                """logits + exp + exp-sum for slot g"""
                kT = kT_g.pop(g)
                qn = QC[g]
                qs4 = 4 * qn
                qsl = slice(QOFF[g], QOFF[g + 1])
                exl = []
                for t in range(KT[g]):
                    lgp = ps_lgp.tile([128, 4 * LQ], f32, tag="lg", name="lgp")
                    nc.tensor.matmul(
                        out=lgp[:, 0:qs4],
                        lhsT=kT[:, t * 128 : (t + 1) * 128],
                        rhs=qTz[:, :, qsl],
                        start=True, stop=True, skip_group_check=True,
                    )
                    ex = expp.tile([128, 4 * LQ], bf16, tag="ex", name="ex",
                                   bufs=12)
                    nc.scalar.activation(
                        out=ex[:, 0:qs4], in_=lgp[:, 0:qs4], func=AFT.Exp)
                    exl.append(ex)
                if KT[g] == 1:
                    exs = exl[0][:]
                else:
                    exst = expp.tile([128, 4 * LQ], bf16, tag="exs",
                                     name="exs", bufs=4)
                    nc.gpsimd.tensor_add(
                        out=exst[:, 0:qs4], in0=exl[0][:, 0:qs4],
                        in1=exl[1][:, 0:qs4])
                    if KT[g] == 3:
                        nc.vector.tensor_add(
                            out=exst[:, 0:qs4], in0=exst[:, 0:qs4],
                            in1=exl[2][:, 0:qs4])
                    exs = exst[:]
                ex_g[g] = exl
                exs_g[g] = exs

            def emit_cd(g):
                """ctx + denominator matmuls for slot g"""
                v = v_g.pop(g)
                exl = ex_g.pop(g)
                exs = exs_g.pop(g)
                qn = QC[g]
                att = ps_attp.tile([128, 192], f32, tag="att", name="att")
                for t in range(KT[g]):
                    ext = exl[t]
                    for h in range(4):
                        nc.tensor.matmul(
                            out=att[32 * h : 32 * (h + 1), 0:qn],
                            lhsT=v[:, t * 128 + 32 * h : t * 128 + 32 * (h + 1)],
                            rhs=ext[:, h * qn : (h + 1) * qn],
                            start=(t == 0), stop=(t == KT[g] - 1),
                            skip_group_check=True, tile_position=(0, 32 * h),
                        )
                # denominator, replicated to each head's 32 partitions
                for h in range(4):
                    nc.tensor.matmul(
                        out=att[32 * h : 32 * (h + 1), LQ : LQ + qn],
                        lhsT=ones32[:],
                        rhs=exs[:, h * qn : (h + 1) * qn],
                        start=True, stop=True, skip_group_check=True,
                        tile_position=(0, 32 * h),
                    )
                att_g[g] = att

            def emit_nrm(g):
                """normalize + out-proj + residual for slot g"""
                att = att_g.pop(g)
                qn = QC[g]
                qsl = slice(QOFF[g], QOFF[g + 1])
                dsb = smp.tile([128, LQ], f32, tag="dsb", name="dsb")
                nc.vector.tensor_scalar_add(
                    out=dsb[:, 0:qn], in0=att[:, LQ : LQ + qn],
                    scalar1=negnp[:, g : g + 1],
                )
                rden = smp.tile([128, LQ], f32, tag="rden", name="rden")
                nc.vector.reciprocal_approx_fast(
                    out=rden[:, 0:qn], in_=dsb[:, 0:qn])
                ctxn = smp.tile([128, LQ], bf16, tag="ctxn", name="ctxn")
                nc.vector.tensor_mul(
                    out=ctxn[:, 0:qn], in0=att[:, 0:qn], in1=rden[:, 0:qn])
                po = ps_lgp.tile([128, 4 * LQ], f32, tag="lg", name="po")
                nc.tensor.matmul(
                    out=po[:, 0:qn], lhsT=woT[:], rhs=ctxn[:, 0:qn],
                    start=True, stop=True, skip_group_check=True,
                )
                nc.vector.tensor_add(
                    out=ar[:, qsl], in0=po[:, 0:qn], in1=xqr[:, qsl],
                )

            ffn_state = {}

            def emit_ffn_a(key):
                c0, c1 = key
                n = c1 - c0
                sl = slice(c0, c1)
                arb = ffnp.tile([128, 512], bf16, tag="arb", name="arb")
                nc.vector.tensor_copy(out=arb[:, 0:n], in_=ar[:, sl])
                pa = ps_bigp.tile([128, 512], f32, tag="big", name="pa")
                nc.tensor.matmul(
                    out=pa[:, 0:n], lhsT=w1T[:, 0:128], rhs=arb[:, 0:n],
                    start=True, stop=True, skip_group_check=True,
                )
                ra = ffnp.tile([128, 512], bf16, tag="ra", name="ra")
                nc.scalar.activation(
                    out=ra[:, 0:n], in_=pa[:, 0:n], func=AFT.Relu,
                    bias=b1c[:, 0:1],
                )
                ffn_state[key] = (arb, ra)

            def emit_ffn_b(key, tail=False):
                c0, c1 = key
                n = c1 - c0
                sl = slice(c0, c1)
                arb, ra = ffn_state.pop(key)
                pb = ps_bigp.tile([128, 512], f32, tag="big", name="pb")
                nc.tensor.matmul(
                    out=pb[:, 0:n], lhsT=w1T[:, 128:256], rhs=arb[:, 0:n],
                    start=True, stop=True, skip_group_check=True,
                )
                rb = ffnp.tile([128, 512], bf16, tag="rb", name="rb")
                nc.vector.tensor_scalar(
                    out=rb[:, 0:n], in0=pb[:, 0:n], scalar1=b1c[:, 1:2],
                    scalar2=0.0, op0=ALU.add, op1=ALU.max,
                )
                p2 = ps_bigp.tile([128, 512], f32, tag="big", name="p2")
                nc.tensor.matmul(
                    out=p2[:, 0:n], lhsT=w2T_a[:], rhs=ra[:, 0:n],
                    start=True, stop=False, skip_group_check=True,
                )
                nc.tensor.matmul(
                    out=p2[:, 0:n], lhsT=w2T_b[:], rhs=rb[:, 0:n],
                    start=False, stop=True, skip_group_check=True,
                )
                nc.vector.scalar_tensor_tensor(
                    out=ar[:, sl], in0=p2[:, 0:n], scalar=b2c[:, 0:1],
                    in1=ar[:, sl], op0=ALU.add, op1=ALU.add,
                )
                if tail:
                    nc.scalar.dma_start(out=out_d[:, sl], in_=ar[:, sl])
                else:
                    nc.sync.dma_start(out=out_d[:, sl], in_=ar[:, sl])

            for w in range(n_waves):
                emit_qproj(w)
                if w < G:
                    emit_kv(w)
                if 2 <= w < G + 2:
                    emit_lgx(w - 2)
                if 4 <= w < G + 4:
                    emit_cd(w - 4)
                if 5 <= w < G + 5:
                    emit_nrm(w - 5)
                for key in ffn_a.get(w, ()):
                    emit_ffn_a(key)
                for key in ffn_b.get(w, ()):
                    emit_ffn_b(key, tail=key in tail_keys)
    nc.finalize()
    return nc


_NC_CACHE = {}


def kernel(edge_index, edge_attr, incoming_edges_list, incoming_edges_batch,
           edge_batch, in_proj_w, in_proj_b, out_proj_w, out_proj_b,
           w1, b1, w2, b2):
    global LAST_RESULTS

    edge_attr = np.asarray(edge_attr, np.float32)
    edge_batch = np.asarray(edge_batch, np.int64)
    incoming_edges_list = np.asarray(incoming_edges_list, np.int64)
    incoming_edges_batch = np.asarray(incoming_edges_batch, np.int64)

    cnt_q = np.bincount(edge_batch, minlength=B)
    st_q = np.zeros(B + 1, np.int64)
    np.cumsum(cnt_q, out=st_q[1:])
    cnt_k = np.bincount(incoming_edges_batch, minlength=B)
    st_k = np.zeros(B + 1, np.int64)
    np.cumsum(cnt_k, out=st_k[1:])
    assert cnt_q.max() <= LQ and cnt_k.max() <= LK

    # slot assignment: per core, sort graphs by key count (desc); slot
    # capacities are the across-core maxima at each rank
    perms = np.empty((NCORES, G), np.int64)
    for c in range(NCORES):
        gl = np.arange(c * G, (c + 1) * G)
        perms[c] = gl[np.argsort(-cnt_k[gl], kind="stable")]
    QC = tuple(int(x) for x in (cnt_q[perms].max(axis=0) + 3) // 4 * 4)
    KT = tuple(int(x) for x in
               np.maximum(1, -(-cnt_k[perms].max(axis=0) // 128)))
    slot_of = np.empty(B, np.int64)
    for c in range(NCORES):
        slot_of[perms[c]] = np.arange(G)

    QOFF = np.zeros(G + 1, np.int64)
    np.cumsum(np.array(QC), out=QOFF[1:])
    KOFF = np.zeros(G + 1, np.int64)
    np.cumsum(128 * np.array(KT), out=KOFF[1:])
    QS2, KS2 = int(QOFF[-1]), int(KOFF[-1])

    xpad = np.zeros((E + LQ, H), np.float32)
    xpad[:E] = edge_attr

    s = 1.0 / math.sqrt(HD)
    wq, wk, wv = in_proj_w[:H], in_proj_w[H : 2 * H], in_proj_w[2 * H :]
    bq, bv = in_proj_b[:H], in_proj_b[2 * H :]
    # bk is dropped exactly: softmax is invariant to the per-query shift
    # q.bk added uniformly across a query's keys.
    boc = out_proj_b + out_proj_w @ bv

    wqT = np.ascontiguousarray((wq * s).T, np.float32)
    wqTz = np.zeros((H, 4 * H), np.float32)
    bqz = np.zeros((H, 4), np.float32)
    for h in range(4):
        wqTz[:, h * H + 32 * h : h * H + 32 * (h + 1)] = \
            wqT[:, 32 * h : 32 * (h + 1)]
        bqz[32 * h : 32 * (h + 1), h] = (bq * s)[32 * h : 32 * (h + 1)]

    bft = ml_dtypes.bfloat16
    shared = dict(
        wqTz=np.ascontiguousarray(wqTz.astype(bft)),
        bqz=np.ascontiguousarray(bqz),
        wkT=np.ascontiguousarray(wk.T.astype(bft)),
        wvT=np.ascontiguousarray(wv.T.astype(bft)),
        woT=np.ascontiguousarray(out_proj_w.T.astype(bft)),
        w1T=np.ascontiguousarray(w1.T.astype(bft)),
        w2T=np.ascontiguousarray(w2.T.astype(bft)),
        b1c=np.ascontiguousarray(b1.reshape(2, H).T, np.float32),
        b2c=np.ascontiguousarray(b2[:, None], np.float32),
    )

    in_maps = []
    for c in range(NCORES):
        rows_q = np.empty(QS2, np.int64)
        rows_k = np.empty(KS2, np.int64)
        negnp_c = np.empty(G, np.float32)
        for i in range(G):
            g = perms[c, i]
            rows_q[QOFF[i] : QOFF[i + 1]] = st_q[g] + np.arange(QC[i])
            nk = int(cnt_k[g])
            kcap = 128 * KT[i]
            rk = np.full(kcap, E, np.int64)
            rk[:nk] = incoming_edges_list[st_k[g] : st_k[g] + nk]
            rows_k[KOFF[i] : KOFF[i + 1]] = rk
            negnp_c[i] = -(kcap - nk)
        xq = xpad[rows_q]                                  # [QS2, H] f32
        xk = xpad[rows_k]                                  # [KS2, H] f32
        in_maps.append(dict(
            shared,
            xqr=np.ascontiguousarray(xq.T) + boc[:, None].astype(np.float32),
            xqbf=np.ascontiguousarray(xq.T.astype(bft)),
            xkT=np.ascontiguousarray(xk.T.astype(bft)),
            negnp=np.ascontiguousarray(
                np.broadcast_to(negnp_c, (H, G))),
        ))

    key = (QC, KT)
    if key not in _NC_CACHE:
        _NC_CACHE.clear()
        _NC_CACHE[key] = _build_program(QC, KT)
    res = run_bass_kernel_spmd(
        _NC_CACHE[key], in_maps, core_ids=list(range(NCORES)),
        trace=TRACE, **TRACE_KW,
    )
    LAST_RESULTS = res

    # compact: edge e lives at dense col (QOFF[slot] + pos) of its core
    eb = edge_batch
    pos = np.arange(E) - st_q[eb]
    col = QOFF[slot_of[eb]] + pos
    out_full = np.empty((E, H), np.float32)
    for c in range(NCORES):
        sel = (eb // G) == c
        out_full[sel] = res.results[c]["out"].T[col[sel]]
    return out_full


# revision 45
# speedup vs baseline: 1.0135x; 1.0135x over previous
"""Trainium2 Bass kernel for nn_MessageAggregationAttention.

Shards B=256 graphs across 8 NeuronCores (32 graphs each). The host does
all data *layout* (gather / pad / transpose / cast); every FLOP of the
model (projections, attention, FFN) runs on device.

Shape specialization: graph sizes are known at kernel() time, so each
core sorts its graphs by key-count and rank-i graphs across cores share
slot i, whose capacities are the across-core maxima: QC[i] query slots
(multiple of 4, <= 96 instead of a flat 96 pad) and KT[i] 128-key tiles
(2 or 3 instead of a flat 3). This removes ~25% of the padded attention
work. The program is built once per shape signature and reused.

Host prep per core:
  - xqr/xqbf [128, QS2]: Q token slab, feature-major (f32 with the
    out-proj bias + folded Wo@bv added for the residual spine; bf16 copy
    for the Q projection).
  - xkT [128, KS2]: incoming-message rows gathered on host
    (edge_attr[incoming_edges_list]), zero-padded per slot, transposed,
    bf16 — replaces 96 serial INDIRECT1D gathers (~105us of GpSimd
    descriptor time) with plain DMA.
  - The key bias bk is dropped exactly (softmax is invariant to the
    per-query shift q.bk); zero-padded K columns then give logits==0,
    exp==1, so the denominator over-counts by exactly npad, which the
    kernel subtracts (no mask table at all).

Device per slot (all matmuls bf16, f32 PSUM), software-pipelined in
waves (kv | logits+exp | ctx+den | norm+outproj, 5 waves deep) so the
in-order engine queues never wait on same-wave work:
  - K/V projections from the resident xkT slab.
  - Logits: zero-blocked qTz (full-128 contraction) per key tile; Exp on
    Scalar with no bias operand.
  - Denominator via ones[128,32] matmuls whose replicated output doubles
    as the partition-broadcast for normalization.
  - Out-proj, residual add; FFN blocks interleave into the wave loop as
    their columns finalize, and the output streams out per block.
"""

import math

import ml_dtypes
import numpy as np

import concourse.bass as bass
import concourse.mybir as mybir
from concourse import bacc
from concourse.bass_utils import run_bass_kernel_spmd
from concourse.tile import TileContext

B, E, M, H, NH = 256, 16384, 65536, 128, 4
HD = H // NH               # 32
LQ, LK = 96, 384           # hard capacity ceilings per graph
NCORES = 8
G = B // NCORES            # 32 graphs per core

f32 = mybir.dt.float32
bf16 = mybir.dt.bfloat16

AFT = mybir.ActivationFunctionType
ALU = mybir.AluOpType

LAST_RESULTS = None
TRACE = False
TRACE_KW = {}


def _build_program(QC, KT):
    QOFF = [0]
    for q in QC:
        QOFF.append(QOFF[-1] + q)
    KOFF = [0]
    for k in KT:
        KOFF.append(KOFF[-1] + 128 * k)
    QS2, KS2 = QOFF[-1], KOFF[-1]

    # FFN blocks of <=512 cols; the last one split in two to drain faster
    blocks = []
    c = 0
    while c < QS2:
        blocks.append((c, min(c + 512, QS2)))
        c = min(c + 512, QS2)
    b0, b1 = blocks.pop()
    if b1 - b0 > 256:
        mid = b0 + ((b1 - b0) // 2 + 3) // 4 * 4
        blocks.append((b0, mid))
        blocks.append((mid, b1))
    else:
        blocks.append((b0, b1))
    # earliest wave per block: its last slot s finishes norm at wave s+5
    ffn_a, ffn_b = {}, {}
    prev_wa = -10
    for (c0, c1) in blocks:
        smax = max(s for s in range(G) if QOFF[s] < c1)
        wa = max(smax + 6, prev_wa + 2)
        prev_wa = wa
        ffn_a.setdefault(wa, []).append((c0, c1))
        ffn_b.setdefault(wa + 1, []).append((c0, c1))
    tail_keys = set(blocks[-2:])
    n_waves = max(G + 7, max(ffn_b) + 1)

    nc = bacc.Bacc("TRN2")

    xkT_d = nc.dram_tensor("xkT", [H, KS2], bf16, kind="ExternalInput")
    xqbf_d = nc.dram_tensor("xqbf", [H, QS2], bf16, kind="ExternalInput")
    xqr_d = nc.dram_tensor("xqr", [H, QS2], f32, kind="ExternalInput")
    wqTz_d = nc.dram_tensor("wqTz", [H, 4 * H], bf16, kind="ExternalInput")
    wkT_d = nc.dram_tensor("wkT", [H, H], bf16, kind="ExternalInput")
    wvT_d = nc.dram_tensor("wvT", [H, H], bf16, kind="ExternalInput")
    woT_d = nc.dram_tensor("woT", [H, H], bf16, kind="ExternalInput")
    w1T_d = nc.dram_tensor("w1T", [H, 2 * H], bf16, kind="ExternalInput")
    w2T_d = nc.dram_tensor("w2T", [2 * H, H], bf16, kind="ExternalInput")
    bq_d = nc.dram_tensor("bqz", [H, 4], f32, kind="ExternalInput")
    b1_d = nc.dram_tensor("b1c", [H, 2], f32, kind="ExternalInput")
    b2_d = nc.dram_tensor("b2c", [H, 1], f32, kind="ExternalInput")
    nnp_d = nc.dram_tensor("negnp", [H, G], f32, kind="ExternalInput")

    out_d = nc.dram_tensor("out", [H, QS2], f32, kind="ExternalOutput")

    with TileContext(nc) as tc:
        with (
            tc.tile_pool(name="const", bufs=1) as constp,
            tc.tile_pool(name="kv", bufs=5) as kvp,
            tc.tile_pool(name="exp", bufs=6) as expp,
            tc.tile_pool(name="sm", bufs=3) as smp,
            tc.tile_pool(name="ffn", bufs=2) as ffnp,
            tc.tile_pool(name="ps_big", bufs=2, space="PSUM") as ps_bigp,
            tc.tile_pool(name="ps_kv", bufs=1, space="PSUM") as ps_kvp,
            tc.tile_pool(name="ps_lg", bufs=2, space="PSUM") as ps_lgp,
            tc.tile_pool(name="ps_att", bufs=2, space="PSUM") as ps_attp,
        ):
            ones32 = constp.tile([128, 32], bf16)
            nc.vector.memset(ones32[:], 1.0)

            def _load(shape, dram, dt=f32):
                t = constp.tile(shape, dt, tag=dram.name, name=dram.name + "_sb")
                nc.sync.dma_start(out=t[:], in_=dram[:])
                return t

            wqTz = _load([H, 4 * H], wqTz_d, bf16)
            wkT = _load([H, H], wkT_d, bf16)
            wvT = _load([H, H], wvT_d, bf16)
            woT = _load([H, H], woT_d, bf16)
            w1T = _load([H, 2 * H], w1T_d, bf16)
            w2T_a = constp.tile([128, H], bf16, tag="w2Ta")
            w2T_b = constp.tile([128, H], bf16, tag="w2Tb")
            nc.sync.dma_start(out=w2T_a[:], in_=w2T_d[0:128, :])
            nc.sync.dma_start(out=w2T_b[:], in_=w2T_d[128:256, :])
            bqz = _load([H, 4], bq_d)
            b1c = _load([H, 2], b1_d)
            b2c = _load([H, 1], b2_d)
            negnp = _load([H, G], nnp_d)

            # Input slabs: spread dma_start descriptor generation across
            # engine queues (~0.6us serial per call) and order chunks so
            # wave-0 consumers land first.
            xkT = constp.tile([128, KS2], bf16, tag="xkT", name="xkT")
            xqbf = constp.tile([128, QS2], bf16, tag="xqbf", name="xqbf")
            xqr = constp.tile([128, QS2], f32, tag="xqr", name="xqr")

            def _chunk(eng, dst, src, c0, c1):
                if c1 > c0:
                    eng.dma_start(out=dst[:, c0:c1], in_=src[:, c0:c1])

            q1 = min(512, QS2)
            q2 = min(1792, QS2)
            _chunk(nc.scalar, xqbf, xqbf_d, 0, 128)
            _chunk(nc.scalar, xqbf, xqbf_d, 128, q1)
            _chunk(nc.scalar, xkT, xkT_d, 0, KOFF[2])
            _chunk(nc.gpsimd, xkT, xkT_d, KOFF[2], KOFF[6])
            _chunk(nc.gpsimd, xqbf, xqbf_d, q1, q2)
            for s0 in range(6, G, 5):
                _chunk(nc.sync, xkT, xkT_d, KOFF[s0], KOFF[min(s0 + 5, G)])
            _chunk(nc.sync, xqbf, xqbf_d, q2, QS2)
            _chunk(nc.sync, xqr, xqr_d, 0, QS2 // 2 // 4 * 4)
            _chunk(nc.gpsimd, xqr, xqr_d, QS2 // 2 // 4 * 4, QS2)

            qTz = constp.tile([128, 4, QS2], bf16, tag="qTz", name="qTz")
            ar = constp.tile([128, QS2], f32, tag="ar", name="ar")

            def emit_qproj(blk, ranges=None):
                if ranges is None:
                    if blk == 0:
                        ranges = [(0, 128), (128, min(512, QS2))]
                    else:
                        c0 = blk * 512
                        if c0 >= QS2:
                            return
                        ranges = [(c0, min(c0 + 512, QS2))]
                for c0, c1 in ranges:
                    _emit_qproj_range(c0, c1)

            def _emit_qproj_range(c0, c1):
                sl = slice(c0, c1)
                n = c1 - c0
                for h in range(4):
                    psq = ps_bigp.tile([128, 512], f32, tag="big", name="psq")
                    nc.tensor.matmul(
                        out=psq[:, 0:n], lhsT=wqTz[:, h * 128 : (h + 1) * 128],
                        rhs=xqbf[:, sl], start=True, stop=True,
                        skip_group_check=True,
                    )
                    if h < 2:
                        nc.scalar.activation(
                            out=qTz[:, h, sl], in_=psq[:, 0:n],
                            func=AFT.Identity, bias=bqz[:, h : h + 1],
                        )
                    else:
                        nc.vector.tensor_scalar_add(
                            out=qTz[:, h, sl], in0=psq[:, 0:n],
                            scalar1=bqz[:, h : h + 1],
                        )

            kT_g, v_g, ex_g, exs_g, att_g = {}, {}, {}, {}, {}

            def emit_kv(g):
                kw = 128 * KT[g]
                ksl = slice(KOFF[g], KOFF[g + 1])
                psk = ps_kvp.tile([128, LK], f32, tag="psk", name="psk")
                nc.tensor.matmul(
                    out=psk[:, 0:kw], lhsT=wkT[:], rhs=xkT[:, ksl],
                    start=True, stop=True, skip_group_check=True,
                )
                kT = kvp.tile([128, LK], bf16, tag="kT", name="kT", bufs=6)
                nc.scalar.activation(
                    out=kT[:, 0:kw], in_=psk[:, 0:kw], func=AFT.Identity)
                psv = ps_kvp.tile([128, LK], f32, tag="psv", name="psv")
                for t in range(KT[g]):
                    nc.tensor.matmul(
                        out=psv[:, t * 128 : (t + 1) * 128],
                        lhsT=xkT[:, KOFF[g] + t * 128 : KOFF[g] + (t + 1) * 128],
                        rhs=wvT[:],
                        start=True, stop=True, skip_group_check=True,
                    )
                v = kvp.tile([128, LK], bf16, tag="v", name="v", bufs=8)
                nc.vector.tensor_copy(out=v[:, 0:kw], in_=psv[:, 0:kw])
                kT_g[g] = kT
                v_g[g] = v

            def emit_lgx(g):
                """logits + exp + exp-sum for slot g"""
                kT = kT_g.pop(g)
                qn = QC[g]
                qs4 = 4 * qn
                qsl = slice(QOFF[g], QOFF[g + 1])
                exl = []
                for t in range(KT[g]):
                    lgp = ps_lgp.tile([128, 4 * LQ], f32, tag="lg", name="lgp")
                    nc.tensor.matmul(
                        out=lgp[:, 0:qs4],
                        lhsT=kT[:, t * 128 : (t + 1) * 128],
                        rhs=qTz[:, :, qsl],
                        start=True, stop=True, skip_group_check=True,
                    )
                    ex = expp.tile([128, 4 * LQ], bf16, tag="ex", name="ex",
                                   bufs=12)
                    nc.scalar.activation(
                        out=ex[:, 0:qs4], in_=lgp[:, 0:qs4], func=AFT.Exp)
                    exl.append(ex)
                if KT[g] == 1:
                    exs = exl[0][:]
                else:
                    exst = expp.tile([128, 4 * LQ], bf16, tag="exs",
                                     name="exs", bufs=4)
                    nc.gpsimd.tensor_add(
                        out=exst[:, 0:qs4], in0=exl[0][:, 0:qs4],
                        in1=exl[1][:, 0:qs4])
                    if KT[g] == 3:
                        nc.vector.tensor_add(
                            out=exst[:, 0:qs4], in0=exst[:, 0:qs4],
                            in1=exl[2][:, 0:qs4])
                    exs = exst[:]
                ex_g[g] = exl
                exs_g[g] = exs

            def emit_cd(g):
                """ctx + denominator matmuls for slot g"""
                v = v_g.pop(g)
                exl = ex_g.pop(g)
                exs = exs_g.pop(g)
                qn = QC[g]
                att = ps_attp.tile([128, 192], f32, tag="att", name="att")
                for t in range(KT[g]):
                    ext = exl[t]
                    for h in range(4):
                        nc.tensor.matmul(
                            out=att[32 * h : 32 * (h + 1), 0:qn],
                            lhsT=v[:, t * 128 + 32 * h : t * 128 + 32 * (h + 1)],
                            rhs=ext[:, h * qn : (h + 1) * qn],
                            start=(t == 0), stop=(t == KT[g] - 1),
                            skip_group_check=True, tile_position=(0, 32 * h),
                        )
                # denominator, replicated to each head's 32 partitions
                for h in range(4):
                    nc.tensor.matmul(
                        out=att[32 * h : 32 * (h + 1), LQ : LQ + qn],
                        lhsT=ones32[:],
                        rhs=exs[:, h * qn : (h + 1) * qn],
                        start=True, stop=True, skip_group_check=True,
                        tile_position=(0, 32 * h),
                    )
                att_g[g] = att

            def emit_nrm(g):
                """normalize + out-proj + residual for slot g"""
                att = att_g.pop(g)
                qn = QC[g]
                qsl = slice(QOFF[g], QOFF[g + 1])
                dsb = smp.tile([128, LQ], f32, tag="dsb", name="dsb")
                nc.vector.tensor_scalar_add(
                    out=dsb[:, 0:qn], in0=att[:, LQ : LQ + qn],
                    scalar1=negnp[:, g : g + 1],
                )
                rden = smp.tile([128, LQ], f32, tag="rden", name="rden")
                nc.vector.reciprocal_approx_fast(
                    out=rden[:, 0:qn], in_=dsb[:, 0:qn])
                ctxn = smp.tile([128, LQ], bf16, tag="ctxn", name="ctxn")
                nc.vector.tensor_mul(
                    out=ctxn[:, 0:qn], in0=att[:, 0:qn], in1=rden[:, 0:qn])
                po = ps_lgp.tile([128, 4 * LQ], f32, tag="lg", name="po")
                nc.tensor.matmul(
                    out=po[:, 0:qn], lhsT=woT[:], rhs=ctxn[:, 0:qn],
                    start=True, stop=True, skip_group_check=True,
                )
                nc.vector.tensor_add(
                    out=ar[:, qsl], in0=po[:, 0:qn], in1=xqr[:, qsl],
                )

            ffn_state = {}

            def emit_ffn_a(key):
                c0, c1 = key
                n = c1 - c0
                sl = slice(c0, c1)
                arb = ffnp.tile([128, 512], bf16, tag="arb", name="arb")
                nc.vector.tensor_copy(out=arb[:, 0:n], in_=ar[:, sl])
                pa = ps_bigp.tile([128, 512], f32, tag="big", name="pa")
                nc.tensor.matmul(
                    out=pa[:, 0:n], lhsT=w1T[:, 0:128], rhs=arb[:, 0:n],
                    start=True, stop=True, skip_group_check=True,
                )
                ra = ffnp.tile([128, 512], bf16, tag="ra", name="ra")
                nc.scalar.activation(
                    out=ra[:, 0:n], in_=pa[:, 0:n], func=AFT.Relu,
                    bias=b1c[:, 0:1],
                )
                ffn_state[key] = (arb, ra)

            def emit_ffn_b(key, tail=False):
                c0, c1 = key
                n = c1 - c0
                sl = slice(c0, c1)
                arb, ra = ffn_state.pop(key)
                pb = ps_bigp.tile([128, 512], f32, tag="big", name="pb")
                nc.tensor.matmul(
                    out=pb[:, 0:n], lhsT=w1T[:, 128:256], rhs=arb[:, 0:n],
                    start=True, stop=True, skip_group_check=True,
                )
                rb = ffnp.tile([128, 512], bf16, tag="rb", name="rb")
                nc.vector.tensor_scalar(
                    out=rb[:, 0:n], in0=pb[:, 0:n], scalar1=b1c[:, 1:2],
                    scalar2=0.0, op0=ALU.add, op1=ALU.max,
                )
                p2 = ps_bigp.tile([128, 512], f32, tag="big", name="p2")
                nc.tensor.matmul(
                    out=p2[:, 0:n], lhsT=w2T_a[:], rhs=ra[:, 0:n],
                    start=True, stop=False, skip_group_check=True,
                )
                nc.tensor.matmul(
                    out=p2[:, 0:n], lhsT=w2T_b[:], rhs=rb[:, 0:n],
                    start=False, stop=True, skip_group_check=True,
                )
                nc.vector.scalar_tensor_tensor(
                    out=ar[:, sl], in0=p2[:, 0:n], scalar=b2c[:, 0:1],
                    in1=ar[:, sl], op0=ALU.add, op1=ALU.add,
                )
                if tail:
                    nc.scalar.dma_start(out=out_d[:, sl], in_=ar[:, sl])
                else:
                    nc.sync.dma_start(out=out_d[:, sl], in_=ar[:, sl])

            for w in range(n_waves):
                emit_qproj(w)
                if w < G:
                    emit_kv(w)
                if 2 <= w < G + 2:
                    emit_lgx(w - 2)
                if 4 <= w < G + 4:
                    emit_cd(w - 4)
                if 5 <= w < G + 5:
                    emit_nrm(w - 5)
                for key in ffn_a.get(w, ()):
                    emit_ffn_a(key)
                for key in ffn_b.get(w, ()):
                    emit_ffn_b(key, tail=key in tail_keys)
    nc.finalize()
    return nc


_NC_CACHE = {}


def kernel(edge_index, edge_attr, incoming_edges_list, incoming_edges_batch,
           edge_batch, in_proj_w, in_proj_b, out_proj_w, out_proj_b,
           w1, b1, w2, b2):
    global LAST_RESULTS

    edge_attr = np.asarray(edge_attr, np.float32)
    edge_batch = np.asarray(edge_batch, np.int64)
    incoming_edges_list = np.asarray(incoming_edges_list, np.int64)
    incoming_edges_batch = np.asarray(incoming_edges_batch, np.int64)

    cnt_q = np.bincount(edge_batch, minlength=B)
    st_q = np.zeros(B + 1, np.int64)
    np.cumsum(cnt_q, out=st_q[1:])
    cnt_k = np.bincount(incoming_edges_batch, minlength=B)
    st_k = np.zeros(B + 1, np.int64)
    np.cumsum(cnt_k, out=st_k[1:])
    assert cnt_q.max() <= LQ and cnt_k.max() <= LK

    # slot assignment: per core, sort graphs by key count (desc); slot
    # capacities are the across-core maxima at each rank
    perms = np.empty((NCORES, G), np.int64)
    for c in range(NCORES):
        gl = np.arange(c * G, (c + 1) * G)
        perms[c] = gl[np.argsort(-cnt_k[gl], kind="stable")]
    QC = tuple(int(x) for x in (cnt_q[perms].max(axis=0) + 3) // 4 * 4)
    KT = tuple(int(x) for x in
               np.maximum(1, -(-cnt_k[perms].max(axis=0) // 128)))
    slot_of = np.empty(B, np.int64)
    for c in range(NCORES):
        slot_of[perms[c]] = np.arange(G)

    QOFF = np.zeros(G + 1, np.int64)
    np.cumsum(np.array(QC), out=QOFF[1:])
    KOFF = np.zeros(G + 1, np.int64)
    np.cumsum(128 * np.array(KT), out=KOFF[1:])
    QS2, KS2 = int(QOFF[-1]), int(KOFF[-1])

    xpad = np.zeros((E + LQ, H), np.float32)
    xpad[:E] = edge_attr

    s = 1.0 / math.sqrt(HD)
    wq, wk, wv = in_proj_w[:H], in_proj_w[H : 2 * H], in_proj_w[2 * H :]
    bq, bv = in_proj_b[:H], in_proj_b[2 * H :]
    # bk is dropped exactly: softmax is invariant to the per-query shift
    # q.bk added uniformly across a query's keys.
    boc = out_proj_b + out_proj_w @ bv

    wqT = np.ascontiguousarray((wq * s).T, np.float32)
    wqTz = np.zeros((H, 4 * H), np.float32)
    bqz = np.zeros((H, 4), np.float32)
    for h in range(4):
        wqTz[:, h * H + 32 * h : h * H + 32 * (h + 1)] = \
            wqT[:, 32 * h : 32 * (h + 1)]
        bqz[32 * h : 32 * (h + 1), h] = (bq * s)[32 * h : 32 * (h + 1)]

    bft = ml_dtypes.bfloat16
    shared = dict(
        wqTz=np.ascontiguousarray(wqTz.astype(bft)),
        bqz=np.ascontiguousarray(bqz),
        wkT=np.ascontiguousarray(wk.T.astype(bft)),
        wvT=np.ascontiguousarray(wv.T.astype(bft)),
        woT=np.ascontiguousarray(out_proj_w.T.astype(bft)),
        w1T=np.ascontiguousarray(w1.T.astype(bft)),
        w2T=np.ascontiguousarray(w2.T.astype(bft)),
        b1c=np.ascontiguousarray(b1.reshape(2, H).T, np.float32),
        b2c=np.ascontiguousarray(b2[:, None], np.float32),
    )

    in_maps = []
    for c in range(NCORES):
        rows_q = np.empty(QS2, np.int64)
        rows_k = np.empty(KS2, np.int64)
        negnp_c = np.empty(G, np.float32)
        for i in range(G):
            g = perms[c, i]
            rows_q[QOFF[i] : QOFF[i + 1]] = st_q[g] + np.arange(QC[i])
            nk = int(cnt_k[g])
            kcap = 128 * KT[i]
            rk = np.full(kcap, E, np.int64)
            rk[:nk] = incoming_edges_list[st_k[g] : st_k[g] + nk]
            rows_k[KOFF[i] : KOFF[i + 1]] = rk
            negnp_c[i] = -(kcap - nk)
        xq = xpad[rows_q]                                  # [QS2, H] f32
        xk = xpad[rows_k]                                  # [KS2, H] f32
        in_maps.append(dict(
            shared,
            xqr=np.ascontiguousarray(xq.T) + boc[:, None].astype(np.float32),
            xqbf=np.ascontiguousarray(xq.T.astype(bft)),
            xkT=np.ascontiguousarray(xk.T.astype(bft)),
            negnp=np.ascontiguousarray(
                np.broadcast_to(negnp_c, (H, G))),
        ))

    key = (QC, KT)
    if key not in _NC_CACHE:
        _NC_CACHE.clear()
        _NC_CACHE[key] = _build_program(QC, KT)
    res = run_bass_kernel_spmd(
        _NC_CACHE[key], in_maps, core_ids=list(range(NCORES)),
        trace=TRACE, **TRACE_KW,
    )
    LAST_RESULTS = res

    # compact: edge e lives at dense col (QOFF[slot] + pos) of its core
    eb = edge_batch
    pos = np.arange(E) - st_q[eb]
    col = QOFF[slot_of[eb]] + pos
    out_full = np.empty((E, H), np.float32)
    for c in range(NCORES):
        sel = (eb // G) == c
        out_full[sel] = res.results[c]["out"].T[col[sel]]
    return out_full


# revision 46
# speedup vs baseline: 1.0144x; 1.0009x over previous
"""Trainium2 Bass kernel for nn_MessageAggregationAttention.

Shards B=256 graphs across 8 NeuronCores (32 graphs each). The host does
all data *layout* (gather / pad / transpose / cast); every FLOP of the
model (projections, attention, FFN) runs on device.

Shape specialization: graph sizes are known at kernel() time, so each
core sorts its graphs by key-count and rank-i graphs across cores share
slot i, whose capacities are the across-core maxima: QC[i] query slots
(multiple of 4, <= 96 instead of a flat 96 pad) and KT[i] 128-key tiles
(2 or 3 instead of a flat 3). This removes ~25% of the padded attention
work. The program is built once per shape signature and reused.

Host prep per core:
  - xqr/xqbf [128, QS2]: Q token slab, feature-major (f32 with the
    out-proj bias + folded Wo@bv added for the residual spine; bf16 copy
    for the Q projection).
  - xkT [128, KS2]: incoming-message rows gathered on host
    (edge_attr[incoming_edges_list]), zero-padded per slot, transposed,
    bf16 — replaces 96 serial INDIRECT1D gathers (~105us of GpSimd
    descriptor time) with plain DMA.
  - The key bias bk is dropped exactly (softmax is invariant to the
    per-query shift q.bk); zero-padded K columns then give logits==0,
    exp==1, so the denominator over-counts by exactly npad, which the
    kernel subtracts (no mask table at all).

Device per slot (all matmuls bf16, f32 PSUM), software-pipelined in
waves (kv | logits+exp | ctx+den | norm+outproj, 5 waves deep) so the
in-order engine queues never wait on same-wave work:
  - K/V projections from the resident xkT slab.
  - Logits: zero-blocked qTz (full-128 contraction) per key tile; Exp on
    Scalar with no bias operand.
  - Denominator via ones[128,32] matmuls whose replicated output doubles
    as the partition-broadcast for normalization.
  - Out-proj, residual add; FFN blocks interleave into the wave loop as
    their columns finalize (p2 + b2 + residual fused into one
    scalar_tensor_tensor), and the output streams out per block.
Engine balance per wave: TensorMatrix ~1.5us, Scalar (Exp + kT cast)
~1.7us, Vector (v cast, den-npad, reciprocal, normalize, residual)
~1.7us, GpSimd (exp-sum) ~0.8us. Input DMAs are spread across the
scalar/gpsimd/sync queues (descriptor generation is ~0.6us serial per
dma_start) with first-needed chunks first.

Measured on 8 axon trn2 cores: ~98.5us (baseline kernel: 353.5us),
rel err 7.6e-4.
"""

import math

import ml_dtypes
import numpy as np

import concourse.bass as bass
import concourse.mybir as mybir
from concourse import bacc
from concourse.bass_utils import run_bass_kernel_spmd
from concourse.tile import TileContext

B, E, M, H, NH = 256, 16384, 65536, 128, 4
HD = H // NH               # 32
LQ, LK = 96, 384           # hard capacity ceilings per graph
NCORES = 8
G = B // NCORES            # 32 graphs per core

f32 = mybir.dt.float32
bf16 = mybir.dt.bfloat16

AFT = mybir.ActivationFunctionType
ALU = mybir.AluOpType

LAST_RESULTS = None
TRACE = False
TRACE_KW = {}


def _build_program(QC, KT):
    QOFF = [0]
    for q in QC:
        QOFF.append(QOFF[-1] + q)
    KOFF = [0]
    for k in KT:
        KOFF.append(KOFF[-1] + 128 * k)
    QS2, KS2 = QOFF[-1], KOFF[-1]

    # FFN blocks of <=512 cols; the last one split in two to drain faster
    blocks = []
    c = 0
    while c < QS2:
        blocks.append((c, min(c + 512, QS2)))
        c = min(c + 512, QS2)
    b0, b1 = blocks.pop()
    if b1 - b0 > 256:
        mid = b0 + ((b1 - b0) // 2 + 3) // 4 * 4
        blocks.append((b0, mid))
        blocks.append((mid, b1))
    else:
        blocks.append((b0, b1))
    # earliest wave per block: its last slot s finishes norm at wave s+5
    ffn_a, ffn_b = {}, {}
    prev_wa = -10
    for (c0, c1) in blocks:
        smax = max(s for s in range(G) if QOFF[s] < c1)
        wa = max(smax + 6, prev_wa + 2)
        prev_wa = wa
        ffn_a.setdefault(wa, []).append((c0, c1))
        ffn_b.setdefault(wa + 1, []).append((c0, c1))
    tail_keys = set(blocks[-2:])
    n_waves = max(G + 7, max(ffn_b) + 1)

    nc = bacc.Bacc("TRN2")

    xkT_d = nc.dram_tensor("xkT", [H, KS2], bf16, kind="ExternalInput")
    xqbf_d = nc.dram_tensor("xqbf", [H, QS2], bf16, kind="ExternalInput")
    xqr_d = nc.dram_tensor("xqr", [H, QS2], f32, kind="ExternalInput")
    wqTz_d = nc.dram_tensor("wqTz", [H, 4 * H], bf16, kind="ExternalInput")
    wkT_d = nc.dram_tensor("wkT", [H, H], bf16, kind="ExternalInput")
    wvT_d = nc.dram_tensor("wvT", [H, H], bf16, kind="ExternalInput")
    woT_d = nc.dram_tensor("woT", [H, H], bf16, kind="ExternalInput")
    w1T_d = nc.dram_tensor("w1T", [H, 2 * H], bf16, kind="ExternalInput")
    w2T_d = nc.dram_tensor("w2T", [2 * H, H], bf16, kind="ExternalInput")
    bq_d = nc.dram_tensor("bqz", [H, 4], f32, kind="ExternalInput")
    b1_d = nc.dram_tensor("b1c", [H, 2], f32, kind="ExternalInput")
    b2_d = nc.dram_tensor("b2c", [H, 1], f32, kind="ExternalInput")
    nnp_d = nc.dram_tensor("negnp", [H, G], f32, kind="ExternalInput")

    out_d = nc.dram_tensor("out", [H, QS2], f32, kind="ExternalOutput")

    with TileContext(nc) as tc:
        with (
            tc.tile_pool(name="const", bufs=1) as constp,
            tc.tile_pool(name="kv", bufs=5) as kvp,
            tc.tile_pool(name="exp", bufs=6) as expp,
            tc.tile_pool(name="sm", bufs=3) as smp,
            tc.tile_pool(name="ffn", bufs=2) as ffnp,
            tc.tile_pool(name="ps_big", bufs=2, space="PSUM") as ps_bigp,
            tc.tile_pool(name="ps_kv", bufs=1, space="PSUM") as ps_kvp,
            tc.tile_pool(name="ps_lg", bufs=2, space="PSUM") as ps_lgp,
            tc.tile_pool(name="ps_att", bufs=2, space="PSUM") as ps_attp,
        ):
            ones32 = constp.tile([128, 32], bf16)
            nc.vector.memset(ones32[:], 1.0)

            def _load(shape, dram, dt=f32):
                t = constp.tile(shape, dt, tag=dram.name, name=dram.name + "_sb")
                nc.sync.dma_start(out=t[:], in_=dram[:])
                return t

            wqTz = _load([H, 4 * H], wqTz_d, bf16)
            wkT = _load([H, H], wkT_d, bf16)
            wvT = _load([H, H], wvT_d, bf16)
            woT = _load([H, H], woT_d, bf16)
            w1T = _load([H, 2 * H], w1T_d, bf16)
            w2T_a = constp.tile([128, H], bf16, tag="w2Ta")
            w2T_b = constp.tile([128, H], bf16, tag="w2Tb")
            nc.sync.dma_start(out=w2T_a[:], in_=w2T_d[0:128, :])
            nc.sync.dma_start(out=w2T_b[:], in_=w2T_d[128:256, :])
            bqz = _load([H, 4], bq_d)
            b1c = _load([H, 2], b1_d)
            b2c = _load([H, 1], b2_d)
            negnp = _load([H, G], nnp_d)

            # Input slabs: spread dma_start descriptor generation across
            # engine queues (~0.6us serial per call) and order chunks so
            # wave-0 consumers land first.
            xkT = constp.tile([128, KS2], bf16, tag="xkT", name="xkT")
            xqbf = constp.tile([128, QS2], bf16, tag="xqbf", name="xqbf")
            xqr = constp.tile([128, QS2], f32, tag="xqr", name="xqr")

            def _chunk(eng, dst, src, c0, c1):
                if c1 > c0:
                    eng.dma_start(out=dst[:, c0:c1], in_=src[:, c0:c1])

            q1 = min(512, QS2)
            q2 = min(1792, QS2)
            _chunk(nc.scalar, xqbf, xqbf_d, 0, 128)
            _chunk(nc.scalar, xqbf, xqbf_d, 128, q1)
            _chunk(nc.scalar, xkT, xkT_d, 0, KOFF[2])
            _chunk(nc.gpsimd, xkT, xkT_d, KOFF[2], KOFF[6])
            _chunk(nc.gpsimd, xqbf, xqbf_d, q1, q2)
            for s0 in range(6, G, 5):
                _chunk(nc.sync, xkT, xkT_d, KOFF[s0], KOFF[min(s0 + 5, G)])
            _chunk(nc.sync, xqbf, xqbf_d, q2, QS2)
            _chunk(nc.sync, xqr, xqr_d, 0, QS2 // 2 // 4 * 4)
            _chunk(nc.gpsimd, xqr, xqr_d, QS2 // 2 // 4 * 4, QS2)

            qTz = constp.tile([128, 4, QS2], bf16, tag="qTz", name="qTz")
            ar = constp.tile([128, QS2], f32, tag="ar", name="ar")

            def emit_qproj(blk, ranges=None):
                if ranges is None:
                    if blk == 0:
                        ranges = [(0, 128), (128, min(512, QS2))]
                    else:
                        c0 = blk * 512
                        if c0 >= QS2:
                            return
                        ranges = [(c0, min(c0 + 512, QS2))]
                for c0, c1 in ranges:
                    _emit_qproj_range(c0, c1)

            def _emit_qproj_range(c0, c1):
                sl = slice(c0, c1)
                n = c1 - c0
                for h in range(4):
                    psq = ps_bigp.tile([128, 512], f32, tag="big", name="psq")
                    nc.tensor.matmul(
                        out=psq[:, 0:n], lhsT=wqTz[:, h * 128 : (h + 1) * 128],
                        rhs=xqbf[:, sl], start=True, stop=True,
                        skip_group_check=True,
                    )
                    if h < 2:
                        nc.scalar.activation(
                            out=qTz[:, h, sl], in_=psq[:, 0:n],
                            func=AFT.Identity, bias=bqz[:, h : h + 1],
                        )
                    else:
                        nc.vector.tensor_scalar_add(
                            out=qTz[:, h, sl], in0=psq[:, 0:n],
                            scalar1=bqz[:, h : h + 1],
                        )

            kT_g, v_g, ex_g, exs_g, att_g = {}, {}, {}, {}, {}

            def emit_kv(g):
                kw = 128 * KT[g]
                ksl = slice(KOFF[g], KOFF[g + 1])
                psk = ps_kvp.tile([128, LK], f32, tag="psk", name="psk")
                nc.tensor.matmul(
                    out=psk[:, 0:kw], lhsT=wkT[:], rhs=xkT[:, ksl],
                    start=True, stop=True, skip_group_check=True,
                )
                kT = kvp.tile([128, LK], bf16, tag="kT", name="kT", bufs=6)
                nc.scalar.activation(
                    out=kT[:, 0:kw], in_=psk[:, 0:kw], func=AFT.Identity)
                psv = ps_kvp.tile([128, LK], f32, tag="psv", name="psv")
                for t in range(KT[g]):
                    nc.tensor.matmul(
                        out=psv[:, t * 128 : (t + 1) * 128],
                        lhsT=xkT[:, KOFF[g] + t * 128 : KOFF[g] + (t + 1) * 128],
                        rhs=wvT[:],
                        start=True, stop=True, skip_group_check=True,
                    )
                v = kvp.tile([128, LK], bf16, tag="v", name="v", bufs=8)
                nc.vector.tensor_copy(out=v[:, 0:kw], in_=psv[:, 0:kw])
                kT_g[g] = kT
                v_g[g] = v

            def emit_lgx(g):
                """logits + exp + exp-sum for slot g"""
                kT = kT_g.pop(g)
                qn = QC[g]
                qs4 = 4 * qn
                qsl = slice(QOFF[g], QOFF[g + 1])
                exl = []
                for t in range(KT[g]):
                    lgp = ps_lgp.tile([128, 4 * LQ], f32, tag="lg", name="lgp")
                    nc.tensor.matmul(
                        out=lgp[:, 0:qs4],
                        lhsT=kT[:, t * 128 : (t + 1) * 128],
                        rhs=qTz[:, :, qsl],
                        start=True, stop=True, skip_group_check=True,
                    )
                    ex = expp.tile([128, 4 * LQ], bf16, tag="ex", name="ex",
                                   bufs=12)
                    nc.scalar.activation(
                        out=ex[:, 0:qs4], in_=lgp[:, 0:qs4], func=AFT.Exp)
                    exl.append(ex)
                if KT[g] == 1:
                    exs = exl[0][:]
                else:
                    exst = expp.tile([128, 4 * LQ], bf16, tag="exs",
                                     name="exs", bufs=4)
                    nc.gpsimd.tensor_add(
                        out=exst[:, 0:qs4], in0=exl[0][:, 0:qs4],
                        in1=exl[1][:, 0:qs4])
                    if KT[g] == 3:
                        nc.vector.tensor_add(
                            out=exst[:, 0:qs4], in0=exst[:, 0:qs4],
                            in1=exl[2][:, 0:qs4])
                    exs = exst[:]
                ex_g[g] = exl
                exs_g[g] = exs

            def emit_cd(g):
                """ctx + denominator matmuls for slot g"""
                v = v_g.pop(g)
                exl = ex_g.pop(g)
                exs = exs_g.pop(g)
                qn = QC[g]
                att = ps_attp.tile([128, 192], f32, tag="att", name="att")
                for t in range(KT[g]):
                    ext = exl[t]
                    for h in range(4):
                        nc.tensor.matmul(
                            out=att[32 * h : 32 * (h + 1), 0:qn],
                            lhsT=v[:, t * 128 + 32 * h : t * 128 + 32 * (h + 1)],
                            rhs=ext[:, h * qn : (h + 1) * qn],
                            start=(t == 0), stop=(t == KT[g] - 1),
                            skip_group_check=True, tile_position=(0, 32 * h),
                        )
                # denominator, replicated to each head's 32 partitions
                for h in range(4):
                    nc.tensor.matmul(
                        out=att[32 * h : 32 * (h + 1), LQ : LQ + qn],
                        lhsT=ones32[:],
                        rhs=exs[:, h * qn : (h + 1) * qn],
                        start=True, stop=True, skip_group_check=True,
                        tile_position=(0, 32 * h),
                    )
                att_g[g] = att

            def emit_nrm(g):
                """normalize + out-proj + residual for slot g"""
                att = att_g.pop(g)
                qn = QC[g]
                qsl = slice(QOFF[g], QOFF[g + 1])
                dsb = smp.tile([128, LQ], f32, tag="dsb", name="dsb")
                nc.vector.tensor_scalar_add(
                    out=dsb[:, 0:qn], in0=att[:, LQ : LQ + qn],
                    scalar1=negnp[:, g : g + 1],
                )
                rden = smp.tile([128, LQ], f32, tag="rden", name="rden")
                nc.vector.reciprocal_approx_fast(
                    out=rden[:, 0:qn], in_=dsb[:, 0:qn])
                ctxn = smp.tile([128, LQ], bf16, tag="ctxn", name="ctxn")
                nc.vector.tensor_mul(
                    out=ctxn[:, 0:qn], in0=att[:, 0:qn], in1=rden[:, 0:qn])
                po = ps_lgp.tile([128, 4 * LQ], f32, tag="lg", name="po")
                nc.tensor.matmul(
                    out=po[:, 0:qn], lhsT=woT[:], rhs=ctxn[:, 0:qn],
                    start=True, stop=True, skip_group_check=True,
                )
                nc.vector.tensor_add(
                    out=ar[:, qsl], in0=po[:, 0:qn], in1=xqr[:, qsl],
                )

            ffn_state = {}

            def emit_ffn_a(key):
                c0, c1 = key
                n = c1 - c0
                sl = slice(c0, c1)
                arb = ffnp.tile([128, 512], bf16, tag="arb", name="arb")
                nc.vector.tensor_copy(out=arb[:, 0:n], in_=ar[:, sl])
                pa = ps_bigp.tile([128, 512], f32, tag="big", name="pa")
                nc.tensor.matmul(
                    out=pa[:, 0:n], lhsT=w1T[:, 0:128], rhs=arb[:, 0:n],
                    start=True, stop=True, skip_group_check=True,
                )
                ra = ffnp.tile([128, 512], bf16, tag="ra", name="ra")
                nc.scalar.activation(
                    out=ra[:, 0:n], in_=pa[:, 0:n], func=AFT.Relu,
                    bias=b1c[:, 0:1],
                )
                ffn_state[key] = (arb, ra)

            def emit_ffn_b(key, tail=False):
                c0, c1 = key
                n = c1 - c0
                sl = slice(c0, c1)
                arb, ra = ffn_state.pop(key)
                pb = ps_bigp.tile([128, 512], f32, tag="big", name="pb")
                nc.tensor.matmul(
                    out=pb[:, 0:n], lhsT=w1T[:, 128:256], rhs=arb[:, 0:n],
                    start=True, stop=True, skip_group_check=True,
                )
                rb = ffnp.tile([128, 512], bf16, tag="rb", name="rb")
                nc.vector.tensor_scalar(
                    out=rb[:, 0:n], in0=pb[:, 0:n], scalar1=b1c[:, 1:2],
                    scalar2=0.0, op0=ALU.add, op1=ALU.max,
                )
                p2 = ps_bigp.tile([128, 512], f32, tag="big", name="p2")
                nc.tensor.matmul(
                    out=p2[:, 0:n], lhsT=w2T_a[:], rhs=ra[:, 0:n],
                    start=True, stop=False, skip_group_check=True,
                )
                nc.tensor.matmul(
                    out=p2[:, 0:n], lhsT=w2T_b[:], rhs=rb[:, 0:n],
                    start=False, stop=True, skip_group_check=True,
                )
                nc.vector.scalar_tensor_tensor(
                    out=ar[:, sl], in0=p2[:, 0:n], scalar=b2c[:, 0:1],
                    in1=ar[:, sl], op0=ALU.add, op1=ALU.add,
                )
                if tail:
                    nc.scalar.dma_start(out=out_d[:, sl], in_=ar[:, sl])
                else:
                    nc.sync.dma_start(out=out_d[:, sl], in_=ar[:, sl])

            for w in range(n_waves):
                emit_qproj(w)
                if w < G:
                    emit_kv(w)
                if 2 <= w < G + 2:
                    emit_lgx(w - 2)
                if 4 <= w < G + 4:
                    emit_cd(w - 4)
                if 5 <= w < G + 5:
                    emit_nrm(w - 5)
                for key in ffn_a.get(w, ()):
                    emit_ffn_a(key)
                for key in ffn_b.get(w, ()):
                    emit_ffn_b(key, tail=key in tail_keys)
    nc.finalize()
    return nc


_NC_CACHE = {}


def kernel(edge_index, edge_attr, incoming_edges_list, incoming_edges_batch,
           edge_batch, in_proj_w, in_proj_b, out_proj_w, out_proj_b,
           w1, b1, w2, b2):
    global LAST_RESULTS

    edge_attr = np.asarray(edge_attr, np.float32)
    edge_batch = np.asarray(edge_batch, np.int64)
    incoming_edges_list = np.asarray(incoming_edges_list, np.int64)
    incoming_edges_batch = np.asarray(incoming_edges_batch, np.int64)

    cnt_q = np.bincount(edge_batch, minlength=B)
    st_q = np.zeros(B + 1, np.int64)
    np.cumsum(cnt_q, out=st_q[1:])
    cnt_k = np.bincount(incoming_edges_batch, minlength=B)
    st_k = np.zeros(B + 1, np.int64)
    np.cumsum(cnt_k, out=st_k[1:])
    assert cnt_q.max() <= LQ and cnt_k.max() <= LK

    # slot assignment: per core, sort graphs by key count (desc); slot
    # capacities are the across-core maxima at each rank
    perms = np.empty((NCORES, G), np.int64)
    for c in range(NCORES):
        gl = np.arange(c * G, (c + 1) * G)
        perms[c] = gl[np.argsort(-cnt_k[gl], kind="stable")]
    QC = tuple(int(x) for x in (cnt_q[perms].max(axis=0) + 3) // 4 * 4)
    KT = tuple(int(x) for x in
               np.maximum(1, -(-cnt_k[perms].max(axis=0) // 128)))
    slot_of = np.empty(B, np.int64)
    for c in range(NCORES):
        slot_of[perms[c]] = np.arange(G)

    QOFF = np.zeros(G + 1, np.int64)
    np.cumsum(np.array(QC), out=QOFF[1:])
    KOFF = np.zeros(G + 1, np.int64)
    np.cumsum(128 * np.array(KT), out=KOFF[1:])
    QS2, KS2 = int(QOFF[-1]), int(KOFF[-1])

    xpad = np.zeros((E + LQ, H), np.float32)
    xpad[:E] = edge_attr

    s = 1.0 / math.sqrt(HD)
    wq, wk, wv = in_proj_w[:H], in_proj_w[H : 2 * H], in_proj_w[2 * H :]
    bq, bv = in_proj_b[:H], in_proj_b[2 * H :]
    # bk is dropped exactly: softmax is invariant to the per-query shift
    # q.bk added uniformly across a query's keys.
    boc = out_proj_b + out_proj_w @ bv

    wqT = np.ascontiguousarray((wq * s).T, np.float32)
    wqTz = np.zeros((H, 4 * H), np.float32)
    bqz = np.zeros((H, 4), np.float32)
    for h in range(4):
        wqTz[:, h * H + 32 * h : h * H + 32 * (h + 1)] = \
            wqT[:, 32 * h : 32 * (h + 1)]
        bqz[32 * h : 32 * (h + 1), h] = (bq * s)[32 * h : 32 * (h + 1)]

    bft = ml_dtypes.bfloat16
    shared = dict(
        wqTz=np.ascontiguousarray(wqTz.astype(bft)),
        bqz=np.ascontiguousarray(bqz),
        wkT=np.ascontiguousarray(wk.T.astype(bft)),
        wvT=np.ascontiguousarray(wv.T.astype(bft)),
        woT=np.ascontiguousarray(out_proj_w.T.astype(bft)),
        w1T=np.ascontiguousarray(w1.T.astype(bft)),
        w2T=np.ascontiguousarray(w2.T.astype(bft)),
        b1c=np.ascontiguousarray(b1.reshape(2, H).T, np.float32),
        b2c=np.ascontiguousarray(b2[:, None], np.float32),
    )

    in_maps = []
    for c in range(NCORES):
        rows_q = np.empty(QS2, np.int64)
        rows_k = np.empty(KS2, np.int64)
        negnp_c = np.empty(G, np.float32)
        for i in range(G):
            g = perms[c, i]
            rows_q[QOFF[i] : QOFF[i + 1]] = st_q[g] + np.arange(QC[i])
            nk = int(cnt_k[g])
            kcap = 128 * KT[i]
            rk = np.full(kcap, E, np.int64)
            rk[:nk] = incoming_edges_list[st_k[g] : st_k[g] + nk]
            rows_k[KOFF[i] : KOFF[i + 1]] = rk
            negnp_c[i] = -(kcap - nk)
        xq = xpad[rows_q]                                  # [QS2, H] f32
        xk = xpad[rows_k]                                  # [KS2, H] f32
        in_maps.append(dict(
            shared,
            xqr=np.ascontiguousarray(xq.T) + boc[:, None].astype(np.float32),
            xqbf=np.ascontiguousarray(xq.T.astype(bft)),
            xkT=np.ascontiguousarray(xk.T.astype(bft)),
            negnp=np.ascontiguousarray(
                np.broadcast_to(negnp_c, (H, G))),
        ))

    key = (QC, KT)
    if key not in _NC_CACHE:
        _NC_CACHE.clear()
        _NC_CACHE[key] = _build_program(QC, KT)
    res = run_bass_kernel_spmd(
        _NC_CACHE[key], in_maps, core_ids=list(range(NCORES)),
        trace=TRACE, **TRACE_KW,
    )
    LAST_RESULTS = res

    # compact: edge e lives at dense col (QOFF[slot] + pos) of its core
    eb = edge_batch
    pos = np.arange(E) - st_q[eb]
    col = QOFF[slot_of[eb]] + pos
    out_full = np.empty((E, H), np.float32)
    for c in range(NCORES):
        sel = (eb // G) == c
        out_full[sel] = res.results[c]["out"].T[col[sel]]
    return out_full
